# revision 2
# baseline (speedup 1.0000x reference)
"""Trainium2 Bass kernel for CrossDecoder kNN-mining margin loss (fp8, v3).

Device mines approximate top candidates for 6000 queries over 30000
candidates (sharded 3750/core over 8 cores) via fp8 E4M3 DoubleRow matmuls:
score(q,j) = 2 q.y_j (ND data dims) + 32 b1_j + b2_j  ~=  2 q.y - (|y|^2-512).
The device emits per-16-candidate-chunk maxima; the host selects top chunks
(chunk POSITION identifies candidates), rescores exactly, computes the loss.

v3 changes (from 208us v2):
  - query dedup: only unique train_ill indices are mined (5438 -> 43 query
    tiles instead of 47), host scatters chunk maxima back.
  - scan rebalance: per query tile, DVE tensor_reduce's 1 PSUM tile (banks
    0-1) directly, ScalarE copies the other 3 tiles to SBUF fp16 TRANSPOSED
    ([w,g] -> [g,w]), so the DVE max tree runs entirely in 2x mode on
    contiguous stride-1 slices (no 1x tensor_reduce tail): 16->8->4->2->1
    pairwise tensor_tensor maxes.  DVE/j ~2.9us, Scalar/j ~2.8us, both under
    the PE's 3.5us (K=512), hiding the scan and freeing PSUM banks sooner
    (v2 stalled PE ~0.66us/j on DVE bank drains).
  - xs (candidates) DMA split into per-f-tile chunks so the first matmuls
    start before the full 2MB load lands; ~26 dummy warmup matmuls run
    during the DMA so the PE HAM clock-gate (1.2GHz cold) is released by
    the time real work starts.
"""

import os
import numpy as np
import ml_dtypes

M_, N_, D_, T_ = 2, 30000, 256, 3000
KD = M_ * D_                   # 512 contraction (data) dims
NCORES = 8
NSHARD = N_ // NCORES          # 3750
GW = 16                        # candidates per chunk
FCH = 480                      # candidate tile width (one PSUM bank, 30 groups)
NFC = 8                        # candidate tiles per core
NPAD = FCH * NFC               # 3840
NGRP = FCH // GW               # 30 chunk maxima per tile
S1 = 32.0                      # bias row 1 scale
CENTER = 512.0                 # |y|^2 centering (cancels in ranking)
QT = 128                       # queries per tile (PSUM partition dim)
QBLK = 4                       # query tiles per DMA block
NWARM = 26                     # PE warmup matmuls (HAM release during DMA)

NKC = int(os.environ.get("KNN_NKC", "2"))   # k-tile pairs: 2 -> K=512 screen
ND = 128 * 2 * NKC - 2                      # data dims used for selection
NSEL = int(os.environ.get("KNN_NSEL", "48" if NKC == 2 else "96"))

_cache = {}


def _build_program(nqt):
    import concourse.bass as bass
    import concourse.tile as tile
    from concourse import bacc, mybir

    dt = mybir.dt
    nc = bacc.Bacc(
        "TRN2", target_bir_lowering=False, debug=False, num_devices=NCORES
    )

    nq = nqt * QT
    nblk = (nqt + QBLK - 1) // QBLK
    xq_d = nc.dram_tensor("xq", [128, 2 * NKC, nq], dt.float8e4,
                          kind="ExternalInput")
    xs_d = nc.dram_tensor("xs", [128, 2 * NKC, NPAD], dt.float8e4,
                          kind="ExternalInput")
    cand_d = nc.dram_tensor("cand", [nblk, 128, QBLK * NFC * NGRP], dt.float16,
                            kind="ExternalOutput")
    warm_d = nc.dram_tensor("warm", [128, 16], dt.float16,
                            kind="ExternalOutput")

    DR = mybir.MatmulPerfMode.DoubleRow

    with tile.TileContext(nc) as tc:
        with (
            tc.tile_pool(name="resident", bufs=1) as res_pool,
            tc.tile_pool(name="xq", bufs=2) as xq_pool,
            tc.tile_pool(name="cand", bufs=2) as cand_pool,
            tc.tile_pool(name="scr", bufs=2) as scr_pool,
            tc.tile_pool(name="psum", bufs=4, space=bass.MemorySpace.PSUM) as psum_pool,
        ):
            # --- PE warmup: dummy matmuls on a memset tile, during input DMA
            wsrc = res_pool.tile([128, 2, 480], dt.float8e4, tag="wsrc")
            nc.gpsimd.memset(wsrc[:, :, :], 1.0)
            wps = psum_pool.tile([128, 2, NGRP, GW], dt.float32, tag="ps",
                                 name="warm",
                                 padded_shape=[None, None, 32, None])
            for w in range(NWARM):
                nc.tensor.matmul(
                    wps[:, w % 2, :, :],
                    lhsT=wsrc[:, :, 0:128], rhs=wsrc[:, :, 0:480],
                    start=True, stop=True, perf_mode=DR,
                )
            wout = res_pool.tile([128, 16], dt.float16, tag="wout")
            nc.scalar.activation(wout[:, :], wps[:, 0, 0, :],
                                 mybir.ActivationFunctionType.Copy)
            nc.sync.dma_start(out=warm_d[:, :], in_=wout[:, :])

            # --- resident candidates, chunked per f-tile so MMs start early
            xs_sb = res_pool.tile([128, 2 * NKC, NPAD], dt.float8e4, tag="xs")
            for f in range(NFC):
                nc.sync.dma_start(out=xs_sb[:, :, f * FCH:(f + 1) * FCH],
                                  in_=xs_d[:, :, f * FCH:(f + 1) * FCH])

            for blk in range(nblk):
                q0 = blk * QBLK * QT
                bqt = min(QBLK, nqt - blk * QBLK)
                xq_sb = xq_pool.tile([128, 2 * NKC, bqt * QT], dt.float8e4,
                                     tag="xq", name="xq_sb")
                nc.sync.dma_start(out=xq_sb[:, :, :],
                                  in_=xq_d[:, :, q0:q0 + bqt * QT])
                cand_sb = cand_pool.tile([128, bqt, NFC, NGRP], dt.float16,
                                         tag="cand")
                for j in range(bqt):
                    ps = [psum_pool.tile([128, 2, NGRP, GW], dt.float32,
                                         tag="ps", name=f"ps{p}",
                                         padded_shape=[None, None, 32, None])
                          for p in range(4)]
                    for kc in range(NKC):
                        for f in range(NFC):
                            nc.tensor.matmul(
                                ps[f // 2][:, f % 2, :, :],
                                lhsT=xq_sb[:, 2 * kc:2 * kc + 2,
                                           j * QT:(j + 1) * QT],
                                rhs=xs_sb[:, 2 * kc:2 * kc + 2,
                                          f * FCH:(f + 1) * FCH],
                                start=(kc == 0),
                                stop=(kc == NKC - 1),
                                perf_mode=DR,
                            )
                    # tile p0 (banks 0-1): direct DVE segmented reduce
                    nc.vector.tensor_reduce(
                        cand_sb[:, j, 0:2, :], ps[0][:, :, :, :],
                        axis=mybir.AxisListType.X, op=mybir.AluOpType.max,
                    )
                    # tiles p1..p3: ScalarE transposed copy PSUM->SBUF fp16;
                    # chunk axis w lands at stride NGRP so the whole max tree
                    # runs as contiguous 2x tensor_tensor ops.
                    scr = scr_pool.tile([128, 3, 2, GW, NGRP], dt.float16,
                                        tag="scr")
                    for p in range(1, 4):
                        nc.scalar.activation(
                            scr[:, p - 1, :, :, :].rearrange(
                                "p a w g -> p a g w"),
                            ps[p][:, :, :, :],
                            mybir.ActivationFunctionType.Copy,
                        )
                    t1 = scr_pool.tile([128, 3, 2, 8, NGRP], dt.float16,
                                       tag="t1")
                    nc.vector.tensor_tensor(
                        t1[:, :, :, :, :], scr[:, :, :, 0:8, :],
                        scr[:, :, :, 8:16, :], mybir.AluOpType.max)
                    t2 = scr_pool.tile([128, 3, 2, 4, NGRP], dt.float16,
                                       tag="t2")
                    nc.vector.tensor_tensor(
                        t2[:, :, :, :, :], t1[:, :, :, 0:4, :],
                        t1[:, :, :, 4:8, :], mybir.AluOpType.max)
                    t3 = scr_pool.tile([128, 3, 2, 2, NGRP], dt.float16,
                                       tag="t3")
                    nc.vector.tensor_tensor(
                        t3[:, :, :, :, :], t2[:, :, :, 0:2, :],
                        t2[:, :, :, 2:4, :], mybir.AluOpType.max)
                    nc.vector.tensor_tensor(
                        cand_sb[:, j, 2:8, :].rearrange(
                            "p (t a) g -> p t a g", t=3),
                        t3[:, :, :, 0, :], t3[:, :, :, 1, :],
                        mybir.AluOpType.max)
                nc.sync.dma_start(out=cand_d[blk, :, :bqt * NFC * NGRP],
                                  in_=cand_sb[:, :, :, :])

    nc.compile()
    return nc


def _get_program(nqt):
    key = ("nc", nqt, NKC)
    if key not in _cache:
        _cache[key] = _build_program(nqt)
    return _cache[key]


def _f8(a):
    return np.clip(np.asarray(a, np.float32), -240, 240).astype(
        ml_dtypes.float8_e4m3)


def _prep_inputs(X, uq, nq):
    """X: [N, 512] fp32; uq: unique query ids (len <= nq)."""
    kd = 2 * NKC * 128
    Qm = np.zeros((nq, kd), np.float32)
    nu = len(uq)
    Qm[:nu, :ND] = 2.0 * X[uq, :ND]
    Qm[:nu, ND] = S1
    Qm[:nu, ND + 1] = 1.0
    xq = np.ascontiguousarray(
        _f8(Qm).reshape(nq, 2 * NKC, 128).transpose(2, 1, 0))

    sqy = (X.astype(np.float64) ** 2).sum(1).astype(np.float32)
    bias_t = -(sqy - CENTER)
    b1 = _f8(bias_t / S1).astype(np.float32)
    b2 = _f8(bias_t - S1 * b1).astype(np.float32)

    per_core = []
    for ci in range(NCORES):
        sl = slice(ci * NSHARD, (ci + 1) * NSHARD)
        Z = np.zeros((NPAD, kd), np.float32)
        Z[:NSHARD, :ND] = X[sl, :ND]
        Z[:NSHARD, ND] = b1[sl]
        Z[:NSHARD, ND + 1] = b2[sl]
        Z[NSHARD:, ND:ND + 2] = -240.0    # pad candidates rank last
        xs = np.ascontiguousarray(
            _f8(Z).reshape(NPAD, 2 * NKC, 128).transpose(2, 1, 0))
        per_core.append({"xq": xq, "xs": xs})
    return per_core


def _mine_chunkmax(in_maps, nqt, trace=False):
    from concourse.bass_utils import run_bass_kernel_spmd

    nc = _get_program(nqt)
    try:
        res = run_bass_kernel_spmd(nc, in_maps, list(range(NCORES)), trace=trace)
    except Exception:
        if not trace:
            raise
        res = run_bass_kernel_spmd(nc, in_maps, list(range(NCORES)), trace=False)
    _cache["last_result"] = res
    nblk = (nqt + QBLK - 1) // QBLK
    cores = []
    for i in range(NCORES):
        c = res.results[i]["cand"]                 # [nblk, 128, QBLK*240]
        c = c.reshape(nblk, 128, QBLK, NFC * NGRP).transpose(0, 2, 1, 3)
        cores.append(c.reshape(nblk * QBLK * 128, NFC * NGRP)[:nqt * QT])
    return np.concatenate(cores, axis=1)           # [nq, 1920]


def kernel(outlayer, c, train_ill, k):
    k = int(k)
    outlayer = np.asarray(outlayer, np.float32)
    train_ill = np.asarray(train_ill)
    X = np.ascontiguousarray(
        outlayer.transpose(1, 0, 2).reshape(N_, KD)).astype(np.float32)
    left = train_ill[:, 0].astype(np.int64)
    right = train_ill[:, 1].astype(np.int64)
    q_idx = np.concatenate([right, left])          # [2T]

    uq, inv = np.unique(q_idx, return_inverse=True)
    nqt = max(1, (len(uq) + QT - 1) // QT)
    nq = nqt * QT

    in_maps = _prep_inputs(X, uq, nq)
    cmu = _mine_chunkmax(
        in_maps, nqt, trace=bool(int(os.environ.get("KNN_TRACE", "0"))))
    cm = cmu[inv].astype(np.float32)               # [2T, 1920]

    # top-NSEL chunks per query -> candidate lists with known indices
    top_chunks = np.argpartition(-cm, NSEL, axis=1)[:, :NSEL]
    core = top_chunks // (NPAD // GW)
    jj = top_chunks % (NPAD // GW)
    base = core * NSHARD + jj * GW
    cand = base[:, :, None] + np.arange(GW)[None, None, :]   # [2T, NSEL, 16]
    valid = (jj[:, :, None] * GW + np.arange(GW)[None, None, :]) < NSHARD
    cand = np.where(valid, cand, 0).reshape(2 * T_, NSEL * GW)
    valid = valid.reshape(2 * T_, NSEL * GW)

    # exact rescore (fp32 gather/dot, fp64 assembly)
    nkeep = k + 1
    sq64 = (X.astype(np.float64) ** 2).sum(1)
    B_all = np.zeros((2 * T_, nkeep))
    for q0 in range(0, 2 * T_, 256):
        q1 = min(q0 + 256, 2 * T_)
        qv = X[q_idx[q0:q1]]                                   # [B, 512]
        cv = X[cand[q0:q1]]                                    # [B, C, 512]
        dot = np.matmul(cv, qv[:, :, None].astype(np.float32))[:, :, 0]
        d = (sq64[q_idx[q0:q1], None] + sq64[cand[q0:q1]]
             - 2.0 * dot.astype(np.float64))
        d = np.where(valid[q0:q1], d, np.inf)
        idx = np.argpartition(d, nkeep, axis=1)[:, :nkeep]
        g = X.astype(np.float64)[np.take_along_axis(cand[q0:q1], idx, axis=1)]
        dd = ((qv[:, None, :].astype(np.float64) - g) ** 2).sum(2)
        dd = np.where(np.take_along_axis(valid[q0:q1], idx, axis=1), dd, np.inf)
        B_all[q0:q1] = np.sort(dd, axis=1)
    B2 = B_all[:T_, 1:]            # right-query mining
    B1 = B_all[T_:, 1:]            # left-query mining

    X64 = X.astype(np.float64)
    D = ((X64[left] - X64[right]) ** 2).sum(1) + 1.0
    L1 = np.maximum(D[:, None] - B1, 0.0)
    L2 = np.maximum(D[:, None] - B2, 0.0)
    loss = (L1.mean() + L2.mean()) / 2.0
    return np.asarray(loss, dtype=np.float32)


# revision 3
# speedup vs baseline: 3.9681x; 3.9681x over previous
"""Trainium2 Bass kernel for CrossDecoder kNN-mining margin loss (fp8, v3).

Device mines approximate top candidates for 6000 queries over 30000
candidates (sharded 3750/core over 8 cores) via fp8 E4M3 DoubleRow matmuls:
score(q,j) = 2 q.y_j (ND data dims) + 32 b1_j + b2_j  ~=  2 q.y - (|y|^2-512).
The device emits per-16-candidate-chunk maxima; the host selects top chunks
(chunk POSITION identifies candidates), rescores exactly, computes the loss.

v3 changes (from 208us v2):
  - query dedup: only unique train_ill indices are mined (5438 -> 43 query
    tiles instead of 47), host scatters chunk maxima back.
  - scan rebalance: per query tile, DVE tensor_reduce's 1 PSUM tile (banks
    0-1) directly, ScalarE copies the other 3 tiles to SBUF fp16 TRANSPOSED
    ([w,g] -> [g,w]), so the DVE max tree runs entirely in 2x mode on
    contiguous stride-1 slices (no 1x tensor_reduce tail): 16->8->4->2->1
    pairwise tensor_tensor maxes.  DVE/j ~2.9us, Scalar/j ~2.8us, both under
    the PE's 3.5us (K=512), hiding the scan and freeing PSUM banks sooner
    (v2 stalled PE ~0.66us/j on DVE bank drains).
  - xs (candidates) DMA split into per-f-tile chunks so the first matmuls
    start before the full 2MB load lands; ~26 dummy warmup matmuls run
    during the DMA so the PE HAM clock-gate (1.2GHz cold) is released by
    the time real work starts.
"""

import os
import numpy as np
import ml_dtypes

M_, N_, D_, T_ = 2, 30000, 256, 3000
KD = M_ * D_                   # 512 contraction (data) dims
NCORES = 8
NSHARD = N_ // NCORES          # 3750
GW = 16                        # candidates per chunk
FCH = 480                      # candidate tile width (one PSUM bank, 30 groups)
NFC = 8                        # candidate tiles per core
NPAD = FCH * NFC               # 3840
NGRP = FCH // GW               # 30 chunk maxima per tile
S1 = 32.0                      # bias row 1 scale
CENTER = 512.0                 # |y|^2 centering (cancels in ranking)
QT = 128                       # queries per tile (PSUM partition dim)
QBLK = 4                       # query tiles per DMA block
NWARM = 26                     # PE warmup matmuls (HAM release during DMA)

NKC = int(os.environ.get("KNN_NKC", "2"))   # k-tile pairs: 2 -> K=512 screen
ND = 128 * 2 * NKC - 2                      # data dims used for selection
NSEL = int(os.environ.get("KNN_NSEL", "48" if NKC == 2 else "96"))

_cache = {}


def _build_program(nqt):
    import concourse.bass as bass
    import concourse.tile as tile
    from concourse import bacc, mybir

    dt = mybir.dt
    nc = bacc.Bacc(
        "TRN2", target_bir_lowering=False, debug=False, num_devices=NCORES
    )

    nq = nqt * QT
    nblk = (nqt + QBLK - 1) // QBLK
    xq_d = nc.dram_tensor("xq", [128, 2 * NKC, nq], dt.float8e4,
                          kind="ExternalInput")
    xs_d = nc.dram_tensor("xs", [128, 2 * NKC, NPAD], dt.float8e4,
                          kind="ExternalInput")
    cand_d = nc.dram_tensor("cand", [nblk, 128, QBLK * NFC * NGRP], dt.float16,
                            kind="ExternalOutput")
    warm_d = nc.dram_tensor("warm", [128, 16], dt.float16,
                            kind="ExternalOutput")

    DR = mybir.MatmulPerfMode.DoubleRow

    with tile.TileContext(nc) as tc:
        with (
            tc.tile_pool(name="resident", bufs=1) as res_pool,
            tc.tile_pool(name="xq", bufs=2) as xq_pool,
            tc.tile_pool(name="cand", bufs=2) as cand_pool,
            tc.tile_pool(name="scr", bufs=2) as scr_pool,
            tc.tile_pool(name="psum", bufs=4, space=bass.MemorySpace.PSUM) as psum_pool,
        ):
            # --- PE warmup: dummy matmuls on a memset tile, during input DMA
            wsrc = res_pool.tile([128, 2, 480], dt.float8e4, tag="wsrc")
            nc.gpsimd.memset(wsrc[:, :, :], 1.0)
            wps = psum_pool.tile([128, 2, NGRP, GW], dt.float32, tag="ps",
                                 name="warm",
                                 padded_shape=[None, None, 32, None])
            for w in range(NWARM):
                nc.tensor.matmul(
                    wps[:, w % 2, :, :],
                    lhsT=wsrc[:, :, 0:128], rhs=wsrc[:, :, 0:480],
                    start=True, stop=True, perf_mode=DR,
                )
            wout = res_pool.tile([128, 16], dt.float16, tag="wout")
            nc.scalar.activation(wout[:, :], wps[:, 0, 0, :],
                                 mybir.ActivationFunctionType.Copy)
            nc.sync.dma_start(out=warm_d[:, :], in_=wout[:, :])

            # --- resident candidates, chunked per f-tile so MMs start early
            xs_sb = res_pool.tile([128, 2 * NKC, NPAD], dt.float8e4, tag="xs")
            for f in range(NFC):
                nc.sync.dma_start(out=xs_sb[:, :, f * FCH:(f + 1) * FCH],
                                  in_=xs_d[:, :, f * FCH:(f + 1) * FCH])

            for blk in range(nblk):
                q0 = blk * QBLK * QT
                bqt = min(QBLK, nqt - blk * QBLK)
                xq_sb = xq_pool.tile([128, 2 * NKC, bqt * QT], dt.float8e4,
                                     tag="xq", name="xq_sb")
                nc.sync.dma_start(out=xq_sb[:, :, :],
                                  in_=xq_d[:, :, q0:q0 + bqt * QT])
                cand_sb = cand_pool.tile([128, bqt, NFC, NGRP], dt.float16,
                                         tag="cand")
                for j in range(bqt):
                    ps = [psum_pool.tile([128, 2, NGRP, GW], dt.float32,
                                         tag="ps", name=f"ps{p}",
                                         padded_shape=[None, None, 32, None])
                          for p in range(4)]
                    for kc in range(NKC):
                        for f in range(NFC):
                            nc.tensor.matmul(
                                ps[f // 2][:, f % 2, :, :],
                                lhsT=xq_sb[:, 2 * kc:2 * kc + 2,
                                           j * QT:(j + 1) * QT],
                                rhs=xs_sb[:, 2 * kc:2 * kc + 2,
                                          f * FCH:(f + 1) * FCH],
                                start=(kc == 0),
                                stop=(kc == NKC - 1),
                                perf_mode=DR,
                            )
                    # tile p0 (banks 0-1): direct DVE segmented reduce
                    nc.vector.tensor_reduce(
                        cand_sb[:, j, 0:2, :], ps[0][:, :, :, :],
                        axis=mybir.AxisListType.X, op=mybir.AluOpType.max,
                    )
                    # tiles p1..p3: ScalarE contiguous copy PSUM->SBUF fp16,
                    # then a DVE max tree: stages 16->8->4->2 run in 2x mode,
                    # the final 2->1 is a strided 1x tensor_tensor (cheaper
                    # than a 1x tensor_reduce reading 2x the elements).
                    scr = scr_pool.tile([128, 3, 2, NGRP, GW], dt.float16,
                                        tag="scr")
                    for p in range(1, 4):
                        nc.scalar.activation(
                            scr[:, p - 1, :, :, :], ps[p][:, :, :, :],
                            mybir.ActivationFunctionType.Copy,
                        )
                    t1 = scr_pool.tile([128, 3, 2, NGRP, 8], dt.float16,
                                       tag="t1")
                    nc.vector.tensor_tensor(
                        t1[:, :, :, :, :], scr[:, :, :, :, 0:8],
                        scr[:, :, :, :, 8:16], mybir.AluOpType.max)
                    t2 = scr_pool.tile([128, 3, 2, NGRP, 4], dt.float16,
                                       tag="t2")
                    nc.vector.tensor_tensor(
                        t2[:, :, :, :, :], t1[:, :, :, :, 0:4],
                        t1[:, :, :, :, 4:8], mybir.AluOpType.max)
                    t3 = scr_pool.tile([128, 3, 2, NGRP, 2], dt.float16,
                                       tag="t3")
                    nc.vector.tensor_tensor(
                        t3[:, :, :, :, :], t2[:, :, :, :, 0:2],
                        t2[:, :, :, :, 2:4], mybir.AluOpType.max)
                    nc.vector.tensor_tensor(
                        cand_sb[:, j, 2:8, :].rearrange(
                            "p (t a) g -> p t a g", t=3),
                        t3[:, :, :, :, 0], t3[:, :, :, :, 1],
                        mybir.AluOpType.max)
                nc.sync.dma_start(out=cand_d[blk, :, :bqt * NFC * NGRP],
                                  in_=cand_sb[:, :, :, :])

    nc.compile()
    return nc


def _get_program(nqt):
    key = ("nc", nqt, NKC)
    if key not in _cache:
        _cache[key] = _build_program(nqt)
    return _cache[key]


def _f8(a):
    return np.clip(np.asarray(a, np.float32), -240, 240).astype(
        ml_dtypes.float8_e4m3)


def _prep_inputs(X, uq, nq):
    """X: [N, 512] fp32; uq: unique query ids (len <= nq)."""
    kd = 2 * NKC * 128
    Qm = np.zeros((nq, kd), np.float32)
    nu = len(uq)
    Qm[:nu, :ND] = 2.0 * X[uq, :ND]
    Qm[:nu, ND] = S1
    Qm[:nu, ND + 1] = 1.0
    xq = np.ascontiguousarray(
        _f8(Qm).reshape(nq, 2 * NKC, 128).transpose(2, 1, 0))

    sqy = (X.astype(np.float64) ** 2).sum(1).astype(np.float32)
    bias_t = -(sqy - CENTER)
    b1 = _f8(bias_t / S1).astype(np.float32)
    b2 = _f8(bias_t - S1 * b1).astype(np.float32)

    per_core = []
    for ci in range(NCORES):
        sl = slice(ci * NSHARD, (ci + 1) * NSHARD)
        Z = np.zeros((NPAD, kd), np.float32)
        Z[:NSHARD, :ND] = X[sl, :ND]
        Z[:NSHARD, ND] = b1[sl]
        Z[:NSHARD, ND + 1] = b2[sl]
        Z[NSHARD:, ND:ND + 2] = -240.0    # pad candidates rank last
        xs = np.ascontiguousarray(
            _f8(Z).reshape(NPAD, 2 * NKC, 128).transpose(2, 1, 0))
        per_core.append({"xq": xq, "xs": xs})
    return per_core


def _mine_chunkmax(in_maps, nqt, trace=False):
    from concourse.bass_utils import run_bass_kernel_spmd

    nc = _get_program(nqt)
    try:
        res = run_bass_kernel_spmd(nc, in_maps, list(range(NCORES)), trace=trace)
    except Exception:
        if not trace:
            raise
        res = run_bass_kernel_spmd(nc, in_maps, list(range(NCORES)), trace=False)
    _cache["last_result"] = res
    nblk = (nqt + QBLK - 1) // QBLK
    cores = []
    for i in range(NCORES):
        c = res.results[i]["cand"]                 # [nblk, 128, QBLK*240]
        c = c.reshape(nblk, 128, QBLK, NFC * NGRP).transpose(0, 2, 1, 3)
        cores.append(c.reshape(nblk * QBLK * 128, NFC * NGRP)[:nqt * QT])
    return np.concatenate(cores, axis=1)           # [nq, 1920]


def kernel(outlayer, c, train_ill, k):
    k = int(k)
    outlayer = np.asarray(outlayer, np.float32)
    train_ill = np.asarray(train_ill)
    X = np.ascontiguousarray(
        outlayer.transpose(1, 0, 2).reshape(N_, KD)).astype(np.float32)
    left = train_ill[:, 0].astype(np.int64)
    right = train_ill[:, 1].astype(np.int64)
    q_idx = np.concatenate([right, left])          # [2T]

    uq, inv = np.unique(q_idx, return_inverse=True)
    nqt = max(1, (len(uq) + QT - 1) // QT)
    nq = nqt * QT

    in_maps = _prep_inputs(X, uq, nq)
    cmu = _mine_chunkmax(
        in_maps, nqt, trace=bool(int(os.environ.get("KNN_TRACE", "0"))))
    cm = cmu[inv].astype(np.float32)               # [2T, 1920]

    # top-NSEL chunks per query -> candidate lists with known indices
    top_chunks = np.argpartition(-cm, NSEL, axis=1)[:, :NSEL]
    core = top_chunks // (NPAD // GW)
    jj = top_chunks % (NPAD // GW)
    base = core * NSHARD + jj * GW
    cand = base[:, :, None] + np.arange(GW)[None, None, :]   # [2T, NSEL, 16]
    valid = (jj[:, :, None] * GW + np.arange(GW)[None, None, :]) < NSHARD
    cand = np.where(valid, cand, 0).reshape(2 * T_, NSEL * GW)
    valid = valid.reshape(2 * T_, NSEL * GW)

    # exact rescore (fp32 gather/dot, fp64 assembly)
    nkeep = k + 1
    sq64 = (X.astype(np.float64) ** 2).sum(1)
    B_all = np.zeros((2 * T_, nkeep))
    for q0 in range(0, 2 * T_, 256):
        q1 = min(q0 + 256, 2 * T_)
        qv = X[q_idx[q0:q1]]                                   # [B, 512]
        cv = X[cand[q0:q1]]                                    # [B, C, 512]
        dot = np.matmul(cv, qv[:, :, None].astype(np.float32))[:, :, 0]
        d = (sq64[q_idx[q0:q1], None] + sq64[cand[q0:q1]]
             - 2.0 * dot.astype(np.float64))
        d = np.where(valid[q0:q1], d, np.inf)
        idx = np.argpartition(d, nkeep, axis=1)[:, :nkeep]
        g = X.astype(np.float64)[np.take_along_axis(cand[q0:q1], idx, axis=1)]
        dd = ((qv[:, None, :].astype(np.float64) - g) ** 2).sum(2)
        dd = np.where(np.take_along_axis(valid[q0:q1], idx, axis=1), dd, np.inf)
        B_all[q0:q1] = np.sort(dd, axis=1)
    B2 = B_all[:T_, 1:]            # right-query mining
    B1 = B_all[T_:, 1:]            # left-query mining

    X64 = X.astype(np.float64)
    D = ((X64[left] - X64[right]) ** 2).sum(1) + 1.0
    L1 = np.maximum(D[:, None] - B1, 0.0)
    L2 = np.maximum(D[:, None] - B2, 0.0)
    loss = (L1.mean() + L2.mean()) / 2.0
    return np.asarray(loss, dtype=np.float32)


# revision 7
# speedup vs baseline: 4.1131x; 1.0365x over previous
"""Trainium2 Bass kernel for CrossDecoder kNN-mining margin loss (fp8, v3).

Device mines approximate top candidates for 6000 queries over 30000
candidates (sharded 3750/core over 8 cores) via fp8 E4M3 DoubleRow matmuls:
score(q,j) = 2 q.y_j (ND data dims) + 32 b1_j + b2_j  ~=  2 q.y - (|y|^2-512).
The device emits per-16-candidate-chunk maxima; the host selects top chunks
(chunk POSITION identifies candidates), rescores exactly, computes the loss.

v3 changes (from 208us v2):
  - query dedup: only unique train_ill indices are mined (5438 -> 43 query
    tiles instead of 47), host scatters chunk maxima back.
  - scan rebalance: per query tile, DVE tensor_reduce's 1 PSUM tile (banks
    0-1) directly, ScalarE copies the other 3 tiles to SBUF fp16 TRANSPOSED
    ([w,g] -> [g,w]), so the DVE max tree runs entirely in 2x mode on
    contiguous stride-1 slices (no 1x tensor_reduce tail): 16->8->4->2->1
    pairwise tensor_tensor maxes.  DVE/j ~2.9us, Scalar/j ~2.8us, both under
    the PE's 3.5us (K=512), hiding the scan and freeing PSUM banks sooner
    (v2 stalled PE ~0.66us/j on DVE bank drains).
  - xs (candidates) DMA split into per-f-tile chunks so the first matmuls
    start before the full 2MB load lands; ~26 dummy warmup matmuls run
    during the DMA so the PE HAM clock-gate (1.2GHz cold) is released by
    the time real work starts.
"""

import os
import numpy as np
import ml_dtypes

M_, N_, D_, T_ = 2, 30000, 256, 3000
KD = M_ * D_                   # 512 contraction (data) dims
NCORES = 8
NSHARD = N_ // NCORES          # 3750
GW = 16                        # candidates per chunk
FCH = 480                      # candidate tile width (one PSUM bank, 30 groups)
NFC = 8                        # candidate tiles per core
NPAD = FCH * NFC               # 3840
NGRP = FCH // GW               # 30 chunk maxima per tile
S1 = 32.0                      # bias row 1 scale
CENTER = 512.0                 # |y|^2 centering (cancels in ranking)
QT = 128                       # queries per tile (PSUM partition dim)
QBLK = 4                       # query tiles per DMA block
NWARM = 5                      # PE warmup matmuls (HAM release during DMA)
XCH = 2                        # xs f-tiles per DMA chunk

NKC = int(os.environ.get("KNN_NKC", "2"))   # k-tile pairs: 2 -> K=512 screen
ND = 128 * 2 * NKC - 2                      # data dims used for selection
NSEL = int(os.environ.get("KNN_NSEL", "48" if NKC == 2 else "96"))

_cache = {}


def _build_program(nqt):
    import concourse.bass as bass
    import concourse.tile as tile
    from concourse import bacc, mybir

    dt = mybir.dt
    nc = bacc.Bacc(
        "TRN2", target_bir_lowering=False, debug=False, num_devices=NCORES
    )

    nq = nqt * QT
    nblk = (nqt + QBLK - 1) // QBLK
    xq_d = nc.dram_tensor("xq", [128, 2 * NKC, nq], dt.float8e4,
                          kind="ExternalInput")
    xs_d = nc.dram_tensor("xs", [128, 2 * NKC, NPAD], dt.float8e4,
                          kind="ExternalInput")
    cand_d = nc.dram_tensor("cand", [nblk, 128, QBLK * NFC * NGRP], dt.float16,
                            kind="ExternalOutput")

    DR = mybir.MatmulPerfMode.DoubleRow

    with tile.TileContext(nc) as tc:
        with (
            tc.tile_pool(name="resident", bufs=1) as res_pool,
            tc.tile_pool(name="xq", bufs=2) as xq_pool,
            tc.tile_pool(name="cand", bufs=2) as cand_pool,
            tc.tile_pool(name="scr", bufs=2) as scr_pool,
            tc.tile_pool(name="psum", bufs=4, space=bass.MemorySpace.PSUM) as psum_pool,
        ):
            # --- PE warmup: dummy matmuls on a memset tile, during input DMA;
            # releases the HAM clock gate (1.2GHz cold) before real work.
            wsrc = res_pool.tile([128, 2, 480], dt.float8e4, tag="wsrc")
            nc.vector.memset(wsrc[:, :, :], 1.0)
            wps = psum_pool.tile([128, 2, NGRP, GW], dt.float32, tag="ps",
                                 name="warm",
                                 padded_shape=[None, None, 32, None])
            for w in range(NWARM):
                nc.tensor.matmul(
                    wps[:, w % 2, :, :],
                    lhsT=wsrc[:, :, 0:128], rhs=wsrc[:, :, 0:480],
                    start=True, stop=True, perf_mode=DR,
                )

            # --- resident candidates, chunked so the first matmuls start
            # before the full load lands; first chunk + first query block
            # triggered first on the sync queue.
            xs_sb = res_pool.tile([128, 2 * NKC, NPAD], dt.float8e4, tag="xs")
            xq_tiles = {}

            def xq_dma(blk):
                q0 = blk * QBLK * QT
                bqt = min(QBLK, nqt - blk * QBLK)
                t = xq_pool.tile([128, 2 * NKC, bqt * QT], dt.float8e4,
                                 tag="xq", name="xq_sb")
                nc.sync.dma_start(out=t[:, :, :],
                                  in_=xq_d[:, :, q0:q0 + bqt * QT])
                xq_tiles[blk] = t

            c0 = XCH * FCH
            nc.sync.dma_start(out=xs_sb[:, :, 0:c0], in_=xs_d[:, :, 0:c0])
            xq_dma(0)
            for f0 in range(XCH, NFC, XCH):
                nc.sync.dma_start(
                    out=xs_sb[:, :, f0 * FCH:(f0 + XCH) * FCH],
                    in_=xs_d[:, :, f0 * FCH:(f0 + XCH) * FCH])

            for blk in range(nblk):
                bqt = min(QBLK, nqt - blk * QBLK)
                if blk + 1 < nblk:
                    xq_dma(blk + 1)
                xq_sb = xq_tiles.pop(blk)
                cand_sb = cand_pool.tile([128, bqt, NFC, NGRP], dt.float16,
                                         tag="cand")
                for j in range(bqt):
                    ps = [psum_pool.tile([128, 2, NGRP, GW], dt.float32,
                                         tag="ps", name=f"ps{p}",
                                         padded_shape=[None, None, 32, None])
                          for p in range(4)]
                    for kc in range(NKC):
                        for f in range(NFC):
                            nc.tensor.matmul(
                                ps[f // 2][:, f % 2, :, :],
                                lhsT=xq_sb[:, 2 * kc:2 * kc + 2,
                                           j * QT:(j + 1) * QT],
                                rhs=xs_sb[:, 2 * kc:2 * kc + 2,
                                          f * FCH:(f + 1) * FCH],
                                start=(kc == 0),
                                stop=(kc == NKC - 1),
                                perf_mode=DR,
                            )
                    # tile p0 (banks 0-1): direct DVE segmented reduce
                    nc.vector.tensor_reduce(
                        cand_sb[:, j, 0:2, :], ps[0][:, :, :, :],
                        axis=mybir.AxisListType.X, op=mybir.AluOpType.max,
                    )
                    # tiles p1..p3: ScalarE contiguous copy PSUM->SBUF fp16,
                    # then a DVE max tree: stages 16->8->4->2 run in 2x mode,
                    # the final 2->1 is a strided 1x tensor_tensor (cheaper
                    # than a 1x tensor_reduce reading 2x the elements).
                    scr = scr_pool.tile([128, 3, 2, NGRP, GW], dt.float16,
                                        tag="scr")
                    for p in range(1, 4):
                        nc.scalar.activation(
                            scr[:, p - 1, :, :, :], ps[p][:, :, :, :],
                            mybir.ActivationFunctionType.Copy,
                        )
                    t1 = scr_pool.tile([128, 3, 2, NGRP, 8], dt.float16,
                                       tag="t1")
                    nc.vector.tensor_tensor(
                        t1[:, :, :, :, :], scr[:, :, :, :, 0:8],
                        scr[:, :, :, :, 8:16], mybir.AluOpType.max)
                    t2 = scr_pool.tile([128, 3, 2, NGRP, 4], dt.float16,
                                       tag="t2")
                    nc.vector.tensor_tensor(
                        t2[:, :, :, :, :], t1[:, :, :, :, 0:4],
                        t1[:, :, :, :, 4:8], mybir.AluOpType.max)
                    t3 = scr_pool.tile([128, 3, 2, NGRP, 2], dt.float16,
                                       tag="t3")
                    nc.vector.tensor_tensor(
                        t3[:, :, :, :, :], t2[:, :, :, :, 0:2],
                        t2[:, :, :, :, 2:4], mybir.AluOpType.max)
                    nc.vector.tensor_tensor(
                        cand_sb[:, j, 2:8, :].rearrange(
                            "p (t a) g -> p t a g", t=3),
                        t3[:, :, :, :, 0], t3[:, :, :, :, 1],
                        mybir.AluOpType.max)
                # output DMA on the Scalar queue (HWDGE) so the sync queue
                # never blocks on a cand-ready wait.
                nc.scalar.dma_start(out=cand_d[blk, :, :bqt * NFC * NGRP],
                                    in_=cand_sb[:, :, :, :])

    nc.compile()
    return nc


def _get_program(nqt):
    key = ("nc", nqt, NKC)
    if key not in _cache:
        _cache[key] = _build_program(nqt)
    return _cache[key]


def _f8(a):
    return np.clip(np.asarray(a, np.float32), -240, 240).astype(
        ml_dtypes.float8_e4m3)


def _prep_inputs(X, uq, nq):
    """X: [N, 512] fp32; uq: unique query ids (len <= nq)."""
    kd = 2 * NKC * 128
    Qm = np.zeros((nq, kd), np.float32)
    nu = len(uq)
    Qm[:nu, :ND] = 2.0 * X[uq, :ND]
    Qm[:nu, ND] = S1
    Qm[:nu, ND + 1] = 1.0
    xq = np.ascontiguousarray(
        _f8(Qm).reshape(nq, 2 * NKC, 128).transpose(2, 1, 0))

    sqy = (X.astype(np.float64) ** 2).sum(1).astype(np.float32)
    bias_t = -(sqy - CENTER)
    b1 = _f8(bias_t / S1).astype(np.float32)
    b2 = _f8(bias_t - S1 * b1).astype(np.float32)

    per_core = []
    for ci in range(NCORES):
        sl = slice(ci * NSHARD, (ci + 1) * NSHARD)
        Z = np.zeros((NPAD, kd), np.float32)
        Z[:NSHARD, :ND] = X[sl, :ND]
        Z[:NSHARD, ND] = b1[sl]
        Z[:NSHARD, ND + 1] = b2[sl]
        Z[NSHARD:, ND:ND + 2] = -240.0    # pad candidates rank last
        xs = np.ascontiguousarray(
            _f8(Z).reshape(NPAD, 2 * NKC, 128).transpose(2, 1, 0))
        per_core.append({"xq": xq, "xs": xs})
    return per_core


def _mine_chunkmax(in_maps, nqt, trace=False):
    from concourse.bass_utils import run_bass_kernel_spmd

    nc = _get_program(nqt)
    try:
        res = run_bass_kernel_spmd(nc, in_maps, list(range(NCORES)), trace=trace)
    except Exception:
        if not trace:
            raise
        res = run_bass_kernel_spmd(nc, in_maps, list(range(NCORES)), trace=False)
    _cache["last_result"] = res
    nblk = (nqt + QBLK - 1) // QBLK
    cores = []
    for i in range(NCORES):
        c = res.results[i]["cand"]                 # [nblk, 128, QBLK*240]
        c = c.reshape(nblk, 128, QBLK, NFC * NGRP).transpose(0, 2, 1, 3)
        cores.append(c.reshape(nblk * QBLK * 128, NFC * NGRP)[:nqt * QT])
    return np.concatenate(cores, axis=1)           # [nq, 1920]


def kernel(outlayer, c, train_ill, k):
    k = int(k)
    outlayer = np.asarray(outlayer, np.float32)
    train_ill = np.asarray(train_ill)
    X = np.ascontiguousarray(
        outlayer.transpose(1, 0, 2).reshape(N_, KD)).astype(np.float32)
    left = train_ill[:, 0].astype(np.int64)
    right = train_ill[:, 1].astype(np.int64)
    q_idx = np.concatenate([right, left])          # [2T]

    uq, inv = np.unique(q_idx, return_inverse=True)
    nqt = max(1, (len(uq) + QT - 1) // QT)
    nq = nqt * QT

    in_maps = _prep_inputs(X, uq, nq)
    cmu = _mine_chunkmax(
        in_maps, nqt, trace=bool(int(os.environ.get("KNN_TRACE", "0"))))
    cm = cmu[inv].astype(np.float32)               # [2T, 1920]

    # top-NSEL chunks per query -> candidate lists with known indices
    top_chunks = np.argpartition(-cm, NSEL, axis=1)[:, :NSEL]
    core = top_chunks // (NPAD // GW)
    jj = top_chunks % (NPAD // GW)
    base = core * NSHARD + jj * GW
    cand = base[:, :, None] + np.arange(GW)[None, None, :]   # [2T, NSEL, 16]
    valid = (jj[:, :, None] * GW + np.arange(GW)[None, None, :]) < NSHARD
    cand = np.where(valid, cand, 0).reshape(2 * T_, NSEL * GW)
    valid = valid.reshape(2 * T_, NSEL * GW)

    # exact rescore (fp32 gather/dot, fp64 assembly)
    nkeep = k + 1
    sq64 = (X.astype(np.float64) ** 2).sum(1)
    B_all = np.zeros((2 * T_, nkeep))
    for q0 in range(0, 2 * T_, 256):
        q1 = min(q0 + 256, 2 * T_)
        qv = X[q_idx[q0:q1]]                                   # [B, 512]
        cv = X[cand[q0:q1]]                                    # [B, C, 512]
        dot = np.matmul(cv, qv[:, :, None].astype(np.float32))[:, :, 0]
        d = (sq64[q_idx[q0:q1], None] + sq64[cand[q0:q1]]
             - 2.0 * dot.astype(np.float64))
        d = np.where(valid[q0:q1], d, np.inf)
        idx = np.argpartition(d, nkeep, axis=1)[:, :nkeep]
        g = X.astype(np.float64)[np.take_along_axis(cand[q0:q1], idx, axis=1)]
        dd = ((qv[:, None, :].astype(np.float64) - g) ** 2).sum(2)
        dd = np.where(np.take_along_axis(valid[q0:q1], idx, axis=1), dd, np.inf)
        B_all[q0:q1] = np.sort(dd, axis=1)
    B2 = B_all[:T_, 1:]            # right-query mining
    B1 = B_all[T_:, 1:]            # left-query mining

    X64 = X.astype(np.float64)
    D = ((X64[left] - X64[right]) ** 2).sum(1) + 1.0
    L1 = np.maximum(D[:, None] - B1, 0.0)
    L2 = np.maximum(D[:, None] - B2, 0.0)
    loss = (L1.mean() + L2.mean()) / 2.0
    return np.asarray(loss, dtype=np.float32)


# revision 8
# speedup vs baseline: 4.1234x; 1.0025x over previous
"""Trainium2 Bass kernel for CrossDecoder kNN-mining margin loss (fp8, v3).

Device mines approximate top candidates for 6000 queries over 30000
candidates (sharded 3750/core over 8 cores) via fp8 E4M3 DoubleRow matmuls:
score(q,j) = 2 q.y_j (ND data dims) + 32 b1_j + b2_j  ~=  2 q.y - (|y|^2-512).
The device emits per-16-candidate-chunk maxima; the host selects top chunks
(chunk POSITION identifies candidates), rescores exactly, computes the loss.

v3 changes (from 208us v2):
  - query dedup: only unique train_ill indices are mined (5438 -> 43 query
    tiles instead of 47), host scatters chunk maxima back.
  - scan rebalance: per query tile, DVE tensor_reduce's 1 PSUM tile (banks
    0-1) directly, ScalarE copies the other 3 tiles to SBUF fp16 TRANSPOSED
    ([w,g] -> [g,w]), so the DVE max tree runs entirely in 2x mode on
    contiguous stride-1 slices (no 1x tensor_reduce tail): 16->8->4->2->1
    pairwise tensor_tensor maxes.  DVE/j ~2.9us, Scalar/j ~2.8us, both under
    the PE's 3.5us (K=512), hiding the scan and freeing PSUM banks sooner
    (v2 stalled PE ~0.66us/j on DVE bank drains).
  - xs (candidates) DMA split into per-f-tile chunks so the first matmuls
    start before the full 2MB load lands; ~26 dummy warmup matmuls run
    during the DMA so the PE HAM clock-gate (1.2GHz cold) is released by
    the time real work starts.
"""

import os
import numpy as np
import ml_dtypes

M_, N_, D_, T_ = 2, 30000, 256, 3000
KD = M_ * D_                   # 512 contraction (data) dims
NCORES = 8
NSHARD = N_ // NCORES          # 3750
GW = 16                        # candidates per chunk
FCH = 480                      # candidate tile width (one PSUM bank, 30 groups)
NFC = 8                        # candidate tiles per core
NPAD = FCH * NFC               # 3840
NGRP = FCH // GW               # 30 chunk maxima per tile
S1 = 32.0                      # bias row 1 scale
CENTER = 512.0                 # |y|^2 centering (cancels in ranking)
QT = 128                       # queries per tile (PSUM partition dim)
QBLK = 4                       # query tiles per DMA block
NWARM = 9                      # PE warmup matmuls (HAM release during DMA)
XCH = 2                        # xs f-tiles per DMA chunk

NKC = int(os.environ.get("KNN_NKC", "2"))   # k-tile pairs: 2 -> K=512 screen
ND = 128 * 2 * NKC - 2                      # data dims used for selection
NSEL = int(os.environ.get("KNN_NSEL", "48" if NKC == 2 else "96"))

_cache = {}


def _build_program(nqt):
    import concourse.bass as bass
    import concourse.tile as tile
    from concourse import bacc, mybir

    dt = mybir.dt
    nc = bacc.Bacc(
        "TRN2", target_bir_lowering=False, debug=False, num_devices=NCORES
    )

    nq = nqt * QT
    nblk = (nqt + QBLK - 1) // QBLK
    xq_d = nc.dram_tensor("xq", [128, 2 * NKC, nq], dt.float8e4,
                          kind="ExternalInput")
    xs_d = nc.dram_tensor("xs", [128, 2 * NKC, NPAD], dt.float8e4,
                          kind="ExternalInput")
    cand_d = nc.dram_tensor("cand", [nblk, 128, QBLK * NFC * NGRP], dt.float16,
                            kind="ExternalOutput")

    DR = mybir.MatmulPerfMode.DoubleRow

    with tile.TileContext(nc) as tc:
        with (
            tc.tile_pool(name="resident", bufs=1) as res_pool,
            tc.tile_pool(name="xq", bufs=2) as xq_pool,
            tc.tile_pool(name="cand", bufs=2) as cand_pool,
            tc.tile_pool(name="scr", bufs=2) as scr_pool,
            tc.tile_pool(name="psum", bufs=4, space=bass.MemorySpace.PSUM) as psum_pool,
        ):
            # --- PE warmup: dummy matmuls on a memset tile, during input DMA;
            # releases the HAM clock gate (1.2GHz cold) before real work.
            wsrc = res_pool.tile([128, 2, 480], dt.float8e4, tag="wsrc")
            nc.vector.memset(wsrc[:, :, :], 1.0)
            wps = psum_pool.tile([128, 2, NGRP, GW], dt.float32, tag="ps",
                                 name="warm",
                                 padded_shape=[None, None, 32, None])
            for w in range(NWARM):
                nc.tensor.matmul(
                    wps[:, w % 2, :, :],
                    lhsT=wsrc[:, :, 0:128], rhs=wsrc[:, :, 0:480],
                    start=True, stop=True, perf_mode=DR,
                )

            # --- resident candidates, chunked so the first matmuls start
            # before the full load lands; first chunk + first query block
            # triggered first on the sync queue.
            xs_sb = res_pool.tile([128, 2 * NKC, NPAD], dt.float8e4, tag="xs")
            xq_tiles = {}

            def xq_dma(blk):
                q0 = blk * QBLK * QT
                bqt = min(QBLK, nqt - blk * QBLK)
                t = xq_pool.tile([128, 2 * NKC, bqt * QT], dt.float8e4,
                                 tag="xq", name="xq_sb")
                nc.sync.dma_start(out=t[:, :, :],
                                  in_=xq_d[:, :, q0:q0 + bqt * QT])
                xq_tiles[blk] = t

            c0 = XCH * FCH
            nc.sync.dma_start(out=xs_sb[:, :, 0:c0], in_=xs_d[:, :, 0:c0])
            xq_dma(0)
            for f0 in range(XCH, NFC, XCH):
                nc.sync.dma_start(
                    out=xs_sb[:, :, f0 * FCH:(f0 + XCH) * FCH],
                    in_=xs_d[:, :, f0 * FCH:(f0 + XCH) * FCH])

            for blk in range(nblk):
                bqt = min(QBLK, nqt - blk * QBLK)
                if blk + 1 < nblk:
                    xq_dma(blk + 1)
                xq_sb = xq_tiles.pop(blk)
                cand_sb = cand_pool.tile([128, bqt, NFC, NGRP], dt.float16,
                                         tag="cand")
                for j in range(bqt):
                    ps = [psum_pool.tile([128, 2, NGRP, GW], dt.float32,
                                         tag="ps", name=f"ps{p}",
                                         padded_shape=[None, None, 32, None])
                          for p in range(4)]
                    for kc in range(NKC):
                        for f in range(NFC):
                            nc.tensor.matmul(
                                ps[f // 2][:, f % 2, :, :],
                                lhsT=xq_sb[:, 2 * kc:2 * kc + 2,
                                           j * QT:(j + 1) * QT],
                                rhs=xs_sb[:, 2 * kc:2 * kc + 2,
                                          f * FCH:(f + 1) * FCH],
                                start=(kc == 0),
                                stop=(kc == NKC - 1),
                                perf_mode=DR,
                            )
                    # tile p0 (banks 0-1): direct DVE segmented reduce
                    nc.vector.tensor_reduce(
                        cand_sb[:, j, 0:2, :], ps[0][:, :, :, :],
                        axis=mybir.AxisListType.X, op=mybir.AluOpType.max,
                    )
                    # tiles p1..p3: ScalarE contiguous copy PSUM->SBUF fp16,
                    # then a DVE max tree: stages 16->8->4->2 run in 2x mode,
                    # the final 2->1 is a strided 1x tensor_tensor (cheaper
                    # than a 1x tensor_reduce reading 2x the elements).
                    scr = scr_pool.tile([128, 3, 2, NGRP, GW], dt.float16,
                                        tag="scr")
                    for p in range(1, 4):
                        nc.scalar.activation(
                            scr[:, p - 1, :, :, :], ps[p][:, :, :, :],
                            mybir.ActivationFunctionType.Copy,
                        )
                    t1 = scr_pool.tile([128, 3, 2, NGRP, 8], dt.float16,
                                       tag="t1")
                    nc.vector.tensor_tensor(
                        t1[:, :, :, :, :], scr[:, :, :, :, 0:8],
                        scr[:, :, :, :, 8:16], mybir.AluOpType.max)
                    t2 = scr_pool.tile([128, 3, 2, NGRP, 4], dt.float16,
                                       tag="t2")
                    nc.vector.tensor_tensor(
                        t2[:, :, :, :, :], t1[:, :, :, :, 0:4],
                        t1[:, :, :, :, 4:8], mybir.AluOpType.max)
                    t3 = scr_pool.tile([128, 3, 2, NGRP, 2], dt.float16,
                                       tag="t3")
                    nc.vector.tensor_tensor(
                        t3[:, :, :, :, :], t2[:, :, :, :, 0:2],
                        t2[:, :, :, :, 2:4], mybir.AluOpType.max)
                    nc.vector.tensor_tensor(
                        cand_sb[:, j, 2:8, :].rearrange(
                            "p (t a) g -> p t a g", t=3),
                        t3[:, :, :, :, 0], t3[:, :, :, :, 1],
                        mybir.AluOpType.max)
                # output DMA on the Scalar queue (HWDGE) so the sync queue
                # never blocks on a cand-ready wait.
                nc.scalar.dma_start(out=cand_d[blk, :, :bqt * NFC * NGRP],
                                    in_=cand_sb[:, :, :, :])

    nc.compile()
    return nc


def _get_program(nqt):
    key = ("nc", nqt, NKC)
    if key not in _cache:
        _cache[key] = _build_program(nqt)
    return _cache[key]


def _f8(a):
    return np.clip(np.asarray(a, np.float32), -240, 240).astype(
        ml_dtypes.float8_e4m3)


def _prep_inputs(X, uq, nq):
    """X: [N, 512] fp32; uq: unique query ids (len <= nq)."""
    kd = 2 * NKC * 128
    Qm = np.zeros((nq, kd), np.float32)
    nu = len(uq)
    Qm[:nu, :ND] = 2.0 * X[uq, :ND]
    Qm[:nu, ND] = S1
    Qm[:nu, ND + 1] = 1.0
    xq = np.ascontiguousarray(
        _f8(Qm).reshape(nq, 2 * NKC, 128).transpose(2, 1, 0))

    sqy = (X.astype(np.float64) ** 2).sum(1).astype(np.float32)
    bias_t = -(sqy - CENTER)
    b1 = _f8(bias_t / S1).astype(np.float32)
    b2 = _f8(bias_t - S1 * b1).astype(np.float32)

    per_core = []
    for ci in range(NCORES):
        sl = slice(ci * NSHARD, (ci + 1) * NSHARD)
        Z = np.zeros((NPAD, kd), np.float32)
        Z[:NSHARD, :ND] = X[sl, :ND]
        Z[:NSHARD, ND] = b1[sl]
        Z[:NSHARD, ND + 1] = b2[sl]
        Z[NSHARD:, ND:ND + 2] = -240.0    # pad candidates rank last
        xs = np.ascontiguousarray(
            _f8(Z).reshape(NPAD, 2 * NKC, 128).transpose(2, 1, 0))
        per_core.append({"xq": xq, "xs": xs})
    return per_core


def _mine_chunkmax(in_maps, nqt, trace=False):
    from concourse.bass_utils import run_bass_kernel_spmd

    nc = _get_program(nqt)
    try:
        res = run_bass_kernel_spmd(nc, in_maps, list(range(NCORES)), trace=trace)
    except Exception:
        if not trace:
            raise
        res = run_bass_kernel_spmd(nc, in_maps, list(range(NCORES)), trace=False)
    _cache["last_result"] = res
    nblk = (nqt + QBLK - 1) // QBLK
    cores = []
    for i in range(NCORES):
        c = res.results[i]["cand"]                 # [nblk, 128, QBLK*240]
        c = c.reshape(nblk, 128, QBLK, NFC * NGRP).transpose(0, 2, 1, 3)
        cores.append(c.reshape(nblk * QBLK * 128, NFC * NGRP)[:nqt * QT])
    return np.concatenate(cores, axis=1)           # [nq, 1920]


def kernel(outlayer, c, train_ill, k):
    k = int(k)
    outlayer = np.asarray(outlayer, np.float32)
    train_ill = np.asarray(train_ill)
    X = np.ascontiguousarray(
        outlayer.transpose(1, 0, 2).reshape(N_, KD)).astype(np.float32)
    left = train_ill[:, 0].astype(np.int64)
    right = train_ill[:, 1].astype(np.int64)
    q_idx = np.concatenate([right, left])          # [2T]

    uq, inv = np.unique(q_idx, return_inverse=True)
    nqt = max(1, (len(uq) + QT - 1) // QT)
    nq = nqt * QT

    in_maps = _prep_inputs(X, uq, nq)
    cmu = _mine_chunkmax(
        in_maps, nqt, trace=bool(int(os.environ.get("KNN_TRACE", "0"))))
    cm = cmu[inv].astype(np.float32)               # [2T, 1920]

    # top-NSEL chunks per query -> candidate lists with known indices
    top_chunks = np.argpartition(-cm, NSEL, axis=1)[:, :NSEL]
    core = top_chunks // (NPAD // GW)
    jj = top_chunks % (NPAD // GW)
    base = core * NSHARD + jj * GW
    cand = base[:, :, None] + np.arange(GW)[None, None, :]   # [2T, NSEL, 16]
    valid = (jj[:, :, None] * GW + np.arange(GW)[None, None, :]) < NSHARD
    cand = np.where(valid, cand, 0).reshape(2 * T_, NSEL * GW)
    valid = valid.reshape(2 * T_, NSEL * GW)

    # exact rescore (fp32 gather/dot, fp64 assembly)
    nkeep = k + 1
    sq64 = (X.astype(np.float64) ** 2).sum(1)
    B_all = np.zeros((2 * T_, nkeep))
    for q0 in range(0, 2 * T_, 256):
        q1 = min(q0 + 256, 2 * T_)
        qv = X[q_idx[q0:q1]]                                   # [B, 512]
        cv = X[cand[q0:q1]]                                    # [B, C, 512]
        dot = np.matmul(cv, qv[:, :, None].astype(np.float32))[:, :, 0]
        d = (sq64[q_idx[q0:q1], None] + sq64[cand[q0:q1]]
             - 2.0 * dot.astype(np.float64))
        d = np.where(valid[q0:q1], d, np.inf)
        idx = np.argpartition(d, nkeep, axis=1)[:, :nkeep]
        g = X.astype(np.float64)[np.take_along_axis(cand[q0:q1], idx, axis=1)]
        dd = ((qv[:, None, :].astype(np.float64) - g) ** 2).sum(2)
        dd = np.where(np.take_along_axis(valid[q0:q1], idx, axis=1), dd, np.inf)
        B_all[q0:q1] = np.sort(dd, axis=1)
    B2 = B_all[:T_, 1:]            # right-query mining
    B1 = B_all[T_:, 1:]            # left-query mining

    X64 = X.astype(np.float64)
    D = ((X64[left] - X64[right]) ** 2).sum(1) + 1.0
    L1 = np.maximum(D[:, None] - B1, 0.0)
    L2 = np.maximum(D[:, None] - B2, 0.0)
    loss = (L1.mean() + L2.mean()) / 2.0
    return np.asarray(loss, dtype=np.float32)
